# revision 1
# baseline (speedup 1.0000x reference)
"""Fused multi-head attention (B=4, L=2048, D=1024, H=16) for 8 Trainium2 cores.

Sharding: core c = 2*b + g handles batch b, head-group g (8 heads).
W_q/W_k sliced+row-permuted (RoPE de-interleave) column-parallel, W_o
row-parallel; host sums the two partial outputs per batch (Megatron-style).

Per-core kernel layout:
- Scores computed TRANSPOSED (S_T[ki, qi]) so softmax(P) @ V needs no
  on-chip transpose of P.  Softmax runs without max-subtraction (logits
  bounded for this problem's scale); /sqrt(hd) and +mask fold into the
  Exp activation; the denominator comes from a ones-column in V (M=65).
- Head-PAIR score matmuls are row-tiled: head 2m occupies PE row-groups
  0-1 (partitions 0-63), head 2m+1 groups 2-3 (partitions 64-127); the
  two K=64 matmuls execute concurrently in the array.
- One [128,1024] Exp per (pair, qc, t) covers both heads; ScalarE is the
  ~266us bottleneck and the emission order keeps it streaming: QKV/WO
  projection matmuls are split into ~1us work items pumped into the
  attention t-loops so the PE always has independent work while ACT
  drains the score slabs.
- RoPE is shaped as few full-partition DVE ops (per-op cost depends on
  free size only, not partition count); cs1/cs2 multiplier tables are
  prebuilt on the host.
- Softmax normalization: DVE reciprocal + GpSimd partition_broadcast
  (no broadcast matmuls on the PE).
"""

import sys
from contextlib import ExitStack

import numpy as np

sys.path.insert(0, "/opt/trn_rl_repo")

import ml_dtypes  # noqa: E402

import concourse.bass as bass  # noqa: E402
import concourse.mybir as mybir  # noqa: E402
import concourse.tile as tile  # noqa: E402
from concourse import bacc  # noqa: E402

BF16 = mybir.dt.bfloat16
F32 = mybir.dt.float32
AF = mybir.ActivationFunctionType

B, L, D = 4, 2048, 1024
H, HD = 16, 64
HPC = 8          # heads per core
DH = HPC * HD    # 512 local head dims
NKT = L // 128   # 16 ki tiles
HALF = 1024      # qi half width (rope granularity)


def build_nc(repeats=1):
    nc = bacc.Bacc(
        "TRN2", target_bir_lowering=False, debug=False, enable_asserts=False
    )

    # DRAM I/O (per-core shards, host-prepared layouts)
    xt_d = nc.dram_tensor("xt", [128, 8 * L], BF16, kind="ExternalInput").ap()
    wq_d = nc.dram_tensor("wq", [128, 8 * DH], BF16, kind="ExternalInput").ap()
    wk_d = nc.dram_tensor("wk", [128, 8 * DH], BF16, kind="ExternalInput").ap()
    wv_d = nc.dram_tensor("wv", [128, 8 * DH], BF16, kind="ExternalInput").ap()
    wo_d = nc.dram_tensor("wo", [128, 4 * D], BF16, kind="ExternalInput").ap()
    cs1_d = nc.dram_tensor("cs1T", [128, L], BF16, kind="ExternalInput").ap()
    cs2_d = nc.dram_tensor("cs2T", [128, L], BF16, kind="ExternalInput").ap()
    mask_d = nc.dram_tensor("maskT", [128, NKT], F32, kind="ExternalInput").ap()
    out_d = nc.dram_tensor("out", [L, D], F32, kind="ExternalOutput").ap()
    # cache-busting: the neuron NEFF cache keys on the kernel I/O signature
    # only, so builds with different `repeats` would otherwise silently
    # reuse each other's NEFF (an unused input gets DCE'd; outputs don't)
    rtag_d = nc.dram_tensor(
        "rtag", [1, repeats], F32, kind="ExternalOutput"
    ).ap()

    with tile.TileContext(nc) as tc, ExitStack() as ctx:
        io = ctx.enter_context(tc.tile_pool(name="io", bufs=1))
        prp = ctx.enter_context(tc.tile_pool(name="prp", bufs=2))
        esp = ctx.enter_context(tc.tile_pool(name="esp", bufs=3))
        mis = ctx.enter_context(tc.tile_pool(name="mis", bufs=2))
        pp = ctx.enter_context(tc.tile_pool(name="pp", bufs=1, space="PSUM"))

        # ---- load inputs ----
        xt = io.tile([128, 8 * L], BF16)
        nc.sync.dma_start(xt[:], xt_d)
        wq = io.tile([128, 8 * DH], BF16)
        nc.sync.dma_start(wq[:], wq_d)
        wk = io.tile([128, 8 * DH], BF16)
        nc.sync.dma_start(wk[:], wk_d)
        wv = io.tile([128, 8 * DH], BF16)
        nc.sync.dma_start(wv[:], wv_d)
        wo = io.tile([128, 4 * D], BF16)
        nc.sync.dma_start(wo[:], wo_d)
        csA = io.tile([128, L], BF16)
        nc.sync.dma_start(csA[:], cs1_d)
        csB = io.tile([128, L], BF16)
        nc.sync.dma_start(csB[:], cs2_d)
        maskT = io.tile([128, NKT], F32)
        nc.sync.dma_start(maskT[:], mask_d)
        rt_sb = io.tile([1, repeats], F32)
        nc.vector.memset(rt_sb[:], 0.0)
        nc.sync.dma_start(rtag_d, rt_sb[:])

        # persistent SBUF activations
        q_sb = [io.tile([128, L], BF16, name=f"q_sb{m}") for m in range(4)]
        k_sb = [io.tile([128, L], BF16, name=f"k_sb{m}") for m in range(4)]
        v_sb = [io.tile([128, HPC * 65], BF16, name=f"v_sb{t}") for t in range(NKT)]
        o_sb = [io.tile([128, L], BF16, name=f"o_sb{m}") for m in range(4)]

        def qk_proj_mm(m, half, c, w_sb, pre):
            """8 accumulating MMs -> ps[128,512]; evac to pre[:, 512c] bf16."""
            ps = pp.tile([128, 512], F32, tag="sp", bufs=2, name="ps_proj")
            for k in range(8):
                lhsT = w_sb[:, 512 * k + 128 * m : 512 * k + 128 * m + 128]
                nc.tensor.matmul(
                    ps[:],
                    lhsT,
                    xt[:, 2048 * k + HALF * half + 512 * c :][:, :512],
                    start=(k == 0),
                    stop=(k == 7),
                )
            nc.vector.tensor_copy(pre[:, 512 * c : 512 * (c + 1)], ps[:])

        def rope(half, pre, dst):
            """pre [128,1024] bf16 -> rope -> dst[:, half] bf16.

            rows per 64-block (head): [x0(32) | x1(32)].
            sw = pre with x0/x1 32-row blocks swapped (4 SBUF->SBUF DMAs;
            DVE tensor_tensor requires equal input base partitions, DMA
            crosses partitions for free).
            dst = pre*[c,c,c,c] + sw*[-s,+s,-s,+s]
                = [x0c - x1s | x1c + x0s] per head.
            """
            hs = slice(HALF * half, HALF * (half + 1))
            sw = prp.tile([128, HALF], BF16, tag="sw")
            for b32 in range(4):
                src = 32 * (b32 ^ 1)
                nc.sync.dma_start(
                    sw[32 * b32 : 32 * b32 + 32, :], pre[src : src + 32, :]
                )
            t1 = prp.tile([128, HALF], BF16, tag="t1")
            nc.vector.tensor_mul(t1[:], pre[:], csA[:, hs])
            t2 = prp.tile([128, HALF], BF16, tag="t2")
            nc.vector.tensor_mul(t2[:], sw[:], csB[:, hs])
            nc.vector.tensor_add(dst[:, hs], t1[:], t2[:])

        def v_item(kb, pg):
            """V projection for kb tile, pair-group pg (4 heads), N=256."""
            ps = pp.tile([128, 512], F32, tag="sp", bufs=2, name="ps_v")
            pv = ps[:, 0:256]
            for k in range(8):
                nc.tensor.matmul(
                    pv,
                    xt[:, 2048 * k + 128 * kb : 2048 * k + 128 * (kb + 1)],
                    wv[:, 512 * k + 256 * pg : 512 * k + 256 * (pg + 1)],
                    start=(k == 0),
                    stop=(k == 7),
                )
            v3 = v_sb[kb][:].rearrange("p (h c) -> p h c", c=65)
            hsl = slice(4 * pg, 4 * (pg + 1))
            nc.vector.memset(v3[:, hsl, 64:65], 1.0)
            nc.vector.tensor_copy(
                v3[:, hsl, 0:64], pv.rearrange("p (h c) -> p h c", c=64)
            )

        def outproj_item(qb, c):
            po = pp.tile([128, 512], F32, tag="sp", bufs=2, name="ps_po")
            for dt_ in range(4):
                nc.tensor.matmul(
                    po[:],
                    o_sb[dt_][:, 128 * qb : 128 * (qb + 1)],
                    wo[:, D * dt_ + 512 * c : D * dt_ + 512 * (c + 1)],
                    start=(dt_ == 0),
                    stop=(dt_ == 3),
                )
            ob = mis.tile([128, 512], F32, tag="ob", bufs=3)
            nc.vector.tensor_copy(ob[:], po[:])
            nc.sync.dma_start(
                out_d[128 * qb : 128 * (qb + 1), 512 * c : 512 * (c + 1)],
                ob[:],
            )

        for _rep in range(repeats):
            # ---- projection work items (~1us of PE work each) ----
            def qk_item_list(m, half, w_sb, dst):
                slot = {}

                def mm(c):
                    def f():
                        if "pre" not in slot:
                            slot["pre"] = prp.tile(
                                [128, HALF], BF16, tag="pre", bufs=2, name="pre"
                            )
                        qk_proj_mm(m, half, c, w_sb, slot["pre"])
                    return f

                def rp():
                    def f():
                        rope(half, slot["pre"], dst)
                        slot.clear()
                    return f

                return [mm(0), mm(1), rp()]

            # startup (emitted immediately): K(m0,h0), Q(m0,h0)
            for f in qk_item_list(0, 0, wk, k_sb[0]):
                f()
            for f in qk_item_list(0, 0, wq, q_sb[0]):
                f()

            # deadline-ordered queue of (fn, key); key marks the completion
            # of a dependency group.  Emission order DEFINES dataflow for
            # Tile, so consumers call require() to force-drain producers.
            #  - v(pg0, kb=t) feeds AV of units (p0/p1, qc0) at step t
            #  - K(0,h1) needed by t>=8 of unit (p0,qc0)
            #  - K(m)/Q(m,h0) before unit (pm, qc0); v(pg1) before (p2,qc0)
            #  - Q(*,h1) before sweep qc2
            workq = []

            def add_qk(kind, m, half, w_sb, dst):
                items = qk_item_list(m, half, w_sb, dst)
                workq.extend((f, None) for f in items[:-1])
                workq.append((items[-1], (kind, m, half)))

            def add_v(kb, pg):
                workq.append((lambda: v_item(kb, pg), ("v", pg, kb)))

            add_v(0, 0)
            add_v(1, 0)
            k01 = qk_item_list(0, 1, wk, k_sb[0])
            workq.append((k01[0], None))
            add_v(2, 0)
            workq.append((k01[1], None))
            add_v(3, 0)
            workq.append((k01[2], ("k", 0, 1)))
            for kb in range(4, NKT):
                add_v(kb, 0)
            add_qk("k", 1, 0, wk, k_sb[1])
            add_qk("k", 1, 1, wk, k_sb[1])
            add_qk("q", 1, 0, wq, q_sb[1])
            for kb in range(NKT):
                add_v(kb, 1)
            add_qk("k", 2, 0, wk, k_sb[2])
            add_qk("k", 2, 1, wk, k_sb[2])
            add_qk("q", 2, 0, wq, q_sb[2])
            add_qk("k", 3, 0, wk, k_sb[3])
            add_qk("k", 3, 1, wk, k_sb[3])
            add_qk("q", 3, 0, wq, q_sb[3])
            add_qk("q", 0, 1, wq, q_sb[0])
            add_qk("q", 1, 1, wq, q_sb[1])
            add_qk("q", 2, 1, wq, q_sb[2])
            add_qk("q", 3, 1, wq, q_sb[3])

            wq_pos = [0]
            emitted = {("k", 0, 0), ("q", 0, 0)}

            def _emit_next():
                fn, key = workq[wq_pos[0]]
                fn()
                if key is not None:
                    emitted.add(key)
                wq_pos[0] += 1

            def pump(n=1):
                e = 0
                while e < n and wq_pos[0] < len(workq):
                    _emit_next()
                    e += 1

            def require(key):
                while key not in emitted:
                    assert wq_pos[0] < len(workq), f"missing producer {key}"
                    _emit_next()

            def attn_unit(m, qc):
                qs = slice(512 * qc, 512 * (qc + 1))
                require(("k", m, 0))
                require(("q", m, qc // 2))
                ot = pp.tile([128, 1024], F32, tag="ot", bufs=1, name="ps_ot")
                for t in range(NKT):
                    if t == 8:
                        require(("k", m, 1))
                    st = pp.tile([128, 1024], F32, tag="st2", bufs=2, name="ps_st")
                    for hh in range(2):
                        o = 64 * hh
                        nc.tensor.matmul(
                            st[:, 512 * hh : 512 * (hh + 1)],
                            k_sb[m][o : o + 64, 128 * t : 128 * (t + 1)],
                            q_sb[m][o : o + 64, qs],
                            start=True,
                            stop=True,
                        )
                    es = esp.tile([128, 1024], BF16, tag="es")
                    nc.scalar.activation(
                        es[:], st[:], AF.Exp,
                        bias=maskT[:, t : t + 1], scale=0.125,
                    )
                    pump(3 if t == 0 else 1)
                    require(("v", m // 2, t))
                    for hh in range(2):
                        h = 2 * m + hh
                        nc.tensor.matmul(
                            ot[0:65, 512 * hh : 512 * (hh + 1)],
                            v_sb[t][:, 65 * h : 65 * h + 65],
                            es[:, 512 * hh : 512 * (hh + 1)],
                            start=(t == 0),
                            stop=(t == NKT - 1),
                        )
                # normalize: o = ot[0:64] * bcast(1/ot[64])
                # (partition_broadcast output must start at partition 0 --
                # out base 64 writes garbage on HW)
                for hh in range(2):
                    cseg = slice(512 * hh, 512 * (hh + 1))
                    rec = mis.tile([1, 512], F32, tag="rec", bufs=2)
                    nc.vector.reciprocal(rec[:], ot[64:65, cseg])
                    bcs = mis.tile([64, 512], F32, tag="bcs", bufs=2)
                    nc.gpsimd.partition_broadcast(bcs[:], rec[0:1, :])
                    rows = slice(64 * hh, 64 * (hh + 1))
                    nc.vector.tensor_mul(
                        o_sb[m][rows, qs], ot[0:64, cseg], bcs[:]
                    )

            for qc in range(4):
                for m in range(4):
                    attn_unit(m, qc)
                for qb in range(4 * qc, 4 * qc + 4):
                    for c in range(2):
                        workq.append(
                            (lambda qb=qb, c=c: outproj_item(qb, c), None)
                        )
            pump(10**9)
    nc.compile()
    return nc


def _prep_core_inputs(x, cs1T, cs2T, mask, W_q, W_k, W_v, W_o, b, g):
    bf = ml_dtypes.bfloat16
    gs = slice(g * DH, (g + 1) * DH)

    # RoPE de-interleave row permutation within the head-group slice
    j = np.arange(64)
    perm64 = np.where(j < 32, 2 * j, 2 * (j - 32) + 1)
    perm = (np.arange(HPC)[:, None] * 64 + perm64[None, :]).reshape(-1) + g * DH

    def wtile(wT):  # [1024, 512] -> [128, 8*512] (k-tile k at cols 512k)
        return np.ascontiguousarray(
            wT.reshape(8, 128, DH).transpose(1, 0, 2).reshape(128, 8 * DH)
        ).astype(bf)

    xt = np.ascontiguousarray(
        x[b].T.reshape(8, 128, L).transpose(1, 0, 2).reshape(128, 8 * L)
    ).astype(bf)
    wq = wtile(W_q[perm].T)
    wk = wtile(W_k[perm].T)
    wv = wtile(W_v[gs].T)
    wo = np.ascontiguousarray(
        W_o[:, gs].T.reshape(4, 128, D).transpose(1, 0, 2).reshape(128, 4 * D)
    ).astype(bf)
    return {
        "xt": xt, "wq": wq, "wk": wk, "wv": wv, "wo": wo,
        "cs1T": cs1T, "cs2T": cs2T,
        "maskT": np.ascontiguousarray(mask[b].reshape(NKT, 128).T).astype(
            np.float32
        ),
    }


def make_in_maps(x, freqs_cos, freqs_sin, attention_mask, W_q, W_k, W_v, W_o):
    bf = ml_dtypes.bfloat16
    x = np.asarray(x, np.float32)
    cosT = np.asarray(freqs_cos, np.float32).T  # [32, L]
    sinT = np.asarray(freqs_sin, np.float32).T
    # rope multiplier tables matching q/k tile rows [x0(32)|x1(32)] per head:
    # dst = pre*csA + swapped(pre)*csB
    cs1T = np.ascontiguousarray(np.vstack([cosT] * 4)).astype(bf)
    cs2T = np.ascontiguousarray(
        np.vstack([-sinT, sinT, -sinT, sinT])
    ).astype(bf)
    mask = np.asarray(attention_mask, np.float32)
    W_q, W_k = np.asarray(W_q, np.float32), np.asarray(W_k, np.float32)
    W_v, W_o = np.asarray(W_v, np.float32), np.asarray(W_o, np.float32)
    return [
        _prep_core_inputs(x, cs1T, cs2T, mask, W_q, W_k, W_v, W_o, c // 2, c % 2)
        for c in range(8)
    ]


_CACHE = {}


def kernel(x, freqs_cos, freqs_sin, attention_mask, W_q, W_k, W_v, W_o):
    from concourse.bass_utils import run_bass_kernel_spmd

    if "nc" not in _CACHE:
        _CACHE["nc"] = build_nc()
    nc = _CACHE["nc"]
    in_maps = make_in_maps(
        x, freqs_cos, freqs_sin, attention_mask, W_q, W_k, W_v, W_o
    )
    res = run_bass_kernel_spmd(nc, in_maps, core_ids=list(range(8)))
    outs = [r["out"] for r in res.results]
    full = np.stack([outs[2 * b] + outs[2 * b + 1] for b in range(B)], axis=0)
    return full.astype(np.float32)


if __name__ == "__main__":
    nc = build_nc()
    print("built ok")

